# revision 18
# baseline (speedup 1.0000x reference)
"""Trainium2 Bass kernel for nn_Classifier (GNN edge-MLP link predictor).

Computes, for E candidate edges:
    out[e] = W2 . relu( x_nc[i0[e]] @ W1[:H] + x_pr[i1[e]] @ W1[H:] + b1 ) + b2

Measured bottleneck of the naive kernel: Pool-engine (SWDGE) descriptor
generation for dma_gather costs ~9-10 ns/index in every mode, and the naive
kernel pays 2 gathered indices per edge.  This kernel pays only 1:

  - fc1 is hoisted to the node tables: A = Xnc @ W1[:H], B = Xpr @ W1[H:]
    are computed on device (157 small matmuls each) and kept in SBUF
    node-major as [128 lanes, 157 blocks, 128 feat] bf16.
  - Edges are sharded contiguously across 8 cores.  Per core, the host
    sorts its edges by i0 block (128-node blocks -> 157 buckets, padded to
    static per-bucket caps so the compiled program is input-independent).
  - A-side "gather" is a matmul: stationary = A_block, moving = a
    host-built one-hot (derived from indices only), PSUM gets A[i0[e]]
    feature-major.  ~1 PE cycle per edge instead of a Pool descriptor.
  - B-side is a single Pool dma_gather per 4096-edge window (SBUF-source,
    transposed, bf16) - the unavoidable 1 index/edge.
  - combine: DVE add (PSUM+SBUF), ACT relu(+b1) -> bf16, fc2 matmul
    (stationary w2), DVE/ACT drain (+b2), DMA out.
  - Host maps the bucket-padded column order back to edge order.

All matmuls bf16 with fp32 PSUM.
"""

import numpy as np
import ml_dtypes

import concourse.bass as bass
import concourse.tile as tile
from concourse import bacc, mybir
from concourse import bass_utils

F32 = mybir.dt.float32
BF16 = mybir.dt.bfloat16
I16 = mybir.dt.int16

N_CORES = 8
H = 128
N_NODES = 20000
N_BLOCKS = (N_NODES + 127) // 128  # 157

T_WIN = 2048    # edge columns per gather window
CHUNK = 512     # columns per PSUM tile


def _build(caps: tuple, reps: int = 1):
    """caps[b] = static column capacity of bucket b (multiples of 128,
    summing to a multiple of T_WIN)."""
    cols = int(sum(caps))
    assert cols % T_WIN == 0
    n_win = cols // T_WIN

    # For every CHUNK-wide psum tile, the static list of
    # (lo, hi, block) segments it covers.
    bases = np.concatenate([[0], np.cumsum(caps)]).astype(int)
    segs_per_tile = []
    for t0 in range(0, cols, CHUNK):
        t1 = t0 + CHUNK
        segs = []
        for b in range(len(caps)):
            lo, hi = max(t0, bases[b]), min(t1, bases[b + 1])
            if lo < hi:
                segs.append((lo - t0, hi - t0, b % N_BLOCKS))
        segs_per_tile.append(segs)

    nc = bacc.Bacc("TRN2", target_bir_lowering=False, debug=False,
                   num_devices=N_CORES)

    xnc_t = nc.dram_tensor("xnc_t", [H, N_NODES], BF16, kind="ExternalInput").ap()
    x_pr = nc.dram_tensor("x_pr", [N_NODES, H], BF16, kind="ExternalInput").ap()
    w1 = nc.dram_tensor("w1", [2 * H, H], BF16, kind="ExternalInput").ap()
    b1 = nc.dram_tensor("b1", [H, 1], F32, kind="ExternalInput").ap()
    w2 = nc.dram_tensor("w2", [H, 1], BF16, kind="ExternalInput").ap()
    b2 = nc.dram_tensor("b2", [1, 1], F32, kind="ExternalInput").ap()
    oh = nc.dram_tensor("oh", [128, cols], BF16, kind="ExternalInput").ap()
    idx1 = nc.dram_tensor("idx1", [16, cols // 16], I16, kind="ExternalInput").ap()
    out = nc.dram_tensor("out", [1, cols], F32, kind="ExternalOutput").ap()

    relu = mybir.ActivationFunctionType.Relu
    ident = mybir.ActivationFunctionType.Identity
    add_op = mybir.AluOpType.add

    with tile.TileContext(nc) as tc:
        with (
            tc.tile_pool(name="const", bufs=1) as cpool,
            tc.tile_pool(name="tbl", bufs=1) as tblpool,
            tc.tile_pool(name="xin", bufs=2) as xpool,
            tc.tile_pool(name="idx", bufs=1) as ipool,
            tc.tile_pool(name="ohp", bufs=2) as ohpool,
            tc.tile_pool(name="gat", bufs=3) as gpool,
            tc.tile_pool(name="hbuf", bufs=3) as hpool,
            tc.tile_pool(name="stage", bufs=2) as spool,
            tc.tile_pool(name="psA", bufs=3, space="PSUM") as papool,
            tc.tile_pool(name="psT", bufs=1, space="PSUM") as ptpool,
            tc.tile_pool(name="ps2", bufs=3, space="PSUM") as p2pool,
        ):
            # ---- constants ----
            w1nc = cpool.tile([H, H], BF16, tag="w1nc")
            nc.sync.dma_start(w1nc[:], w1[0:H, :])
            w1pr = cpool.tile([H, H], BF16, tag="w1pr")
            nc.sync.dma_start(w1pr[:], w1[H:2 * H, :])
            b1_sb = cpool.tile([H, 1], F32, tag="b1")
            nc.sync.dma_start(b1_sb[:], b1[:])
            w2_sb = cpool.tile([H, 1], BF16, tag="w2")
            nc.sync.dma_start(w2_sb[:], w2[:])
            b2_sb = cpool.tile([1, 1], F32, tag="b2")
            nc.sync.dma_start(b2_sb[:], b2[:])

            # ---- indices for the B gather ----
            # First window's slice is DMAed separately so gather 0 can
            # start without waiting for the full index replication.
            idx_sb = ipool.tile([128, cols // 16], I16, tag="idx1")
            ic0 = T_WIN // 16
            for k in range(8):
                nc.sync.dma_start(idx_sb[16 * k:16 * (k + 1), 0:ic0],
                                  idx1[:, 0:ic0])
            for k in range(8):
                nc.sync.dma_start(idx_sb[16 * k:16 * (k + 1), ic0:],
                                  idx1[:, ic0:])

            # ---- A node table (ncRNA side, post-fc1) ----
            # node-major [128 lane, block, feat]; node n -> lane n%128,
            # block n//128.  Built as psum = x_chunk^T-stationary @ w1nc.
            a_sb = tblpool.tile([128, N_BLOCKS * H], BF16, tag="a_tbl")
            a_v = a_sb[:].rearrange("p (s f) -> p s f", s=N_BLOCKS)
            # zero first: the last node block is partial, and uninitialized
            # lanes would otherwise feed NaN*0 into matmuls.
            nc.vector.memset(a_sb[:], 0.0)

            for s in range(N_BLOCKS):
                lo = s * 128
                hi = min(N_NODES, lo + 128)
                xs = xpool.tile([H, 128], BF16, tag="xa")
                nc.sync.dma_start(xs[:, 0:hi - lo], xnc_t[:, lo:hi])
                pt = ptpool.tile([128, H], F32, tag="ptbl")
                nc.tensor.matmul(pt[0:hi - lo, :], xs[:, 0:hi - lo],
                                 w1nc[:], start=True, stop=True)
                if s % 2 == 0:
                    nc.scalar.activation(a_v[0:hi - lo, s, :],
                                         pt[0:hi - lo, :], ident)
                else:
                    nc.vector.tensor_scalar_add(a_v[0:hi - lo, s, :],
                                                pt[0:hi - lo, :], 0.0)

            # ---- edge loop ----
            ic = T_WIN // 16
            for _ in range(reps):
                for w in range(n_win):
                    c0 = w * T_WIN
                    gB = gpool.tile([H, T_WIN], BF16, tag="gB")
                    nc.gpsimd.dma_gather(
                        gB[:].rearrange("p (one t) -> p one t", one=1),
                        x_pr,
                        idx_sb[:, w * ic:(w + 1) * ic],
                        T_WIN,
                        T_WIN,
                        H,
                        transpose=True,
                        single_packet=False,
                    )
                    oh_sb = ohpool.tile([128, T_WIN], BF16, tag="oh")
                    nc.sync.dma_start(oh_sb[:], oh[:, c0:c0 + T_WIN])

                    stage = spool.tile([1, T_WIN], F32, tag="stage")
                    for k in range(T_WIN // CHUNK):
                        sl = slice(k * CHUNK, (k + 1) * CHUNK)
                        pa = papool.tile([128, CHUNK], F32, tag="psA")
                        segs = segs_per_tile[(c0 + k * CHUNK) // CHUNK]
                        # B side first: psA = W1pr^T @ x_pr[i1] (full width),
                        # then the A-side one-hot matmuls accumulate onto it.
                        nc.tensor.matmul(
                            pa[:], w1pr[:], gB[:, sl],
                            start=True, stop=False,
                        )
                        for j, (lo, hi, blk) in enumerate(segs):
                            nc.tensor.matmul(
                                pa[:, lo:hi],
                                a_v[:, blk, :],
                                oh_sb[:, k * CHUNK + lo:k * CHUNK + hi],
                                start=False, stop=(j == len(segs) - 1),
                            )
                        # h = relu(psA + b1) -> bf16
                        h = hpool.tile([128, CHUNK], BF16, tag="h")
                        nc.scalar.activation(h[:], pa[:], relu, bias=b1_sb[:])

                        p2 = p2pool.tile([1, CHUNK], F32, tag="ps2")
                        nc.tensor.matmul(p2[:], w2_sb[:], h[:],
                                         start=True, stop=True)
                        if k % 2 == 0:
                            nc.vector.tensor_scalar(
                                stage[:, sl], p2[:], b2_sb[:], None, add_op
                            )
                        else:
                            nc.scalar.activation(stage[:, sl], p2[:], ident,
                                                 bias=b2_sb[:])
                    nc.sync.dma_start(out[:, c0:c0 + T_WIN], stage[:])

    nc.compile()
    return nc


# ---------------------------------------------------------------------------
# Host-side wrapper
# ---------------------------------------------------------------------------

_CACHE: dict = {}


def _get_program(caps):
    key = tuple(caps)
    if key not in _CACHE:
        _CACHE[key] = _build(key)
    return _CACHE[key]


def _wrap16(idx: np.ndarray) -> np.ndarray:
    """int16 [16, n//16] with value i at [i % 16, i // 16]."""
    n = idx.shape[0]
    return np.ascontiguousarray(idx.astype(np.int16).reshape(n // 16, 16).T)


def kernel(
    x_ncRNA: np.ndarray,
    x_Protein: np.ndarray,
    edge_label_index: np.ndarray,
    W1: np.ndarray,
    b1: np.ndarray,
    W2: np.ndarray,
    b2: np.ndarray,
    _trace: bool = False,
) -> np.ndarray:
    E = edge_label_index.shape[1]

    i0 = np.asarray(edge_label_index[0], dtype=np.int64)
    i1 = np.asarray(edge_label_index[1], dtype=np.int64)

    # --- global bucket sort by i0 block, dealt round-robin across cores ---
    # Every bucket's edges are spread evenly over the 8 cores, so the shared
    # per-bucket capacity is ceil(n_b/8) and padding is minimal.
    blk = i0 >> 7
    order = np.argsort(blk, kind="stable")          # global edge ids, sorted
    n_b = np.bincount(blk, minlength=N_BLOCKS)
    bstart = np.concatenate([[0], np.cumsum(n_b)])
    pos = np.arange(E) - np.repeat(bstart[:-1], n_b)  # rank within bucket
    core_of = pos % N_CORES
    rank = pos // N_CORES

    caps = -(-n_b // N_CORES)
    total = int(caps.sum())
    caps[-1] += -total % T_WIN  # keep sum a multiple of T_WIN
    caps = tuple(int(x) for x in caps)
    cols = int(sum(caps))
    bases = np.concatenate([[0], np.cumsum(caps)]).astype(np.int64)
    col = np.repeat(bases[:-1], n_b) + rank         # column for sorted edges

    nc = _get_program(caps)

    xnc_t = np.ascontiguousarray(
        np.asarray(x_ncRNA, np.float32).T.astype(ml_dtypes.bfloat16))
    x_pr = np.ascontiguousarray(
        np.asarray(x_Protein).astype(ml_dtypes.bfloat16))
    w1 = np.ascontiguousarray(np.asarray(W1).astype(ml_dtypes.bfloat16))
    w2 = np.ascontiguousarray(np.asarray(W2).astype(ml_dtypes.bfloat16))
    b1_ = np.ascontiguousarray(np.asarray(b1).reshape(H, 1).astype(np.float32))
    b2_ = np.ascontiguousarray(np.asarray(b2).reshape(1, 1).astype(np.float32))

    in_maps = []
    placement = []  # per core: (global edge ids, their columns)
    for c in range(N_CORES):
        m = core_of == c
        ids_c = order[m]
        cols_c = col[m]

        # one-hot [128, cols] bf16
        oh = np.zeros((128, cols), ml_dtypes.bfloat16)
        oh[i0[ids_c] & 127, cols_c] = 1

        # B-side indices in column order (pad -> node 0)
        idxs = np.zeros(cols, np.int64)
        idxs[cols_c] = i1[ids_c]

        placement.append((ids_c, cols_c))
        in_maps.append({
            "xnc_t": xnc_t, "x_pr": x_pr,
            "w1": w1, "b1": b1_, "w2": w2, "b2": b2_,
            "oh": oh,
            "idx1": _wrap16(idxs),
        })

    res = bass_utils.run_bass_kernel_spmd(
        nc, in_maps, core_ids=list(range(N_CORES)), trace=_trace
    )
    out = np.empty(E, np.float32)
    for c, (ids_c, cols_c) in enumerate(placement):
        out[ids_c] = res.results[c]["out"][0][cols_c]
    kernel._last_results = res
    return out


# revision 19
# speedup vs baseline: 1.0242x; 1.0242x over previous
"""Trainium2 Bass kernel for nn_Classifier (GNN edge-MLP link predictor).

Computes, for E candidate edges:
    out[e] = W2 . relu( x_nc[i0[e]] @ W1[:H] + x_pr[i1[e]] @ W1[H:] + b1 ) + b2

Measured bottleneck of the naive kernel: Pool-engine (SWDGE) descriptor
generation for dma_gather costs ~9-10 ns/index in every mode, and the naive
kernel pays 2 gathered indices per edge.  This kernel pays only 1:

  - fc1 is hoisted to the node tables: A = Xnc @ W1[:H], B = Xpr @ W1[H:]
    are computed on device (157 small matmuls each) and kept in SBUF
    node-major as [128 lanes, 157 blocks, 128 feat] bf16.
  - Edges are sharded contiguously across 8 cores.  Per core, the host
    sorts its edges by i0 block (128-node blocks -> 157 buckets, padded to
    static per-bucket caps so the compiled program is input-independent).
  - A-side "gather" is a matmul: stationary = A_block, moving = a
    host-built one-hot (derived from indices only), PSUM gets A[i0[e]]
    feature-major.  ~1 PE cycle per edge instead of a Pool descriptor.
  - B-side is a single Pool dma_gather per 4096-edge window (SBUF-source,
    transposed, bf16) - the unavoidable 1 index/edge.
  - combine: DVE add (PSUM+SBUF), ACT relu(+b1) -> bf16, fc2 matmul
    (stationary w2), DVE/ACT drain (+b2), DMA out.
  - Host maps the bucket-padded column order back to edge order.

All matmuls bf16 with fp32 PSUM.
"""

import numpy as np
import ml_dtypes

import concourse.bass as bass
import concourse.tile as tile
from concourse import bacc, mybir
from concourse import bass_utils

F32 = mybir.dt.float32
BF16 = mybir.dt.bfloat16
I16 = mybir.dt.int16

N_CORES = 8
H = 128
N_NODES = 20000
N_BLOCKS = (N_NODES + 127) // 128  # 157

T_WIN = 4096    # edge columns per gather window
CHUNK = 512     # columns per PSUM tile


def _build(caps: tuple, reps: int = 1):
    """caps[b] = static column capacity of bucket b (multiples of 128,
    summing to a multiple of T_WIN)."""
    cols = int(sum(caps))
    assert cols % T_WIN == 0
    n_win = cols // T_WIN

    # For every CHUNK-wide psum tile, the static list of
    # (lo, hi, block) segments it covers.
    bases = np.concatenate([[0], np.cumsum(caps)]).astype(int)
    segs_per_tile = []
    for t0 in range(0, cols, CHUNK):
        t1 = t0 + CHUNK
        segs = []
        for b in range(len(caps)):
            lo, hi = max(t0, bases[b]), min(t1, bases[b + 1])
            if lo < hi:
                segs.append((lo - t0, hi - t0, b % N_BLOCKS))
        segs_per_tile.append(segs)

    nc = bacc.Bacc("TRN2", target_bir_lowering=False, debug=False,
                   num_devices=N_CORES)

    xnc_t = nc.dram_tensor("xnc_t", [H, N_NODES], BF16, kind="ExternalInput").ap()
    x_pr = nc.dram_tensor("x_pr", [N_NODES, H], BF16, kind="ExternalInput").ap()
    w1 = nc.dram_tensor("w1", [2 * H, H], BF16, kind="ExternalInput").ap()
    b1 = nc.dram_tensor("b1", [H, 1], F32, kind="ExternalInput").ap()
    w2 = nc.dram_tensor("w2", [H, 1], BF16, kind="ExternalInput").ap()
    b2 = nc.dram_tensor("b2", [1, 1], F32, kind="ExternalInput").ap()
    oh = nc.dram_tensor("oh", [128, cols], BF16, kind="ExternalInput").ap()
    idx1 = nc.dram_tensor("idx1", [16, cols // 16], I16, kind="ExternalInput").ap()
    out = nc.dram_tensor("out", [1, cols], F32, kind="ExternalOutput").ap()

    relu = mybir.ActivationFunctionType.Relu
    ident = mybir.ActivationFunctionType.Identity
    add_op = mybir.AluOpType.add

    with tile.TileContext(nc) as tc:
        with (
            tc.tile_pool(name="const", bufs=1) as cpool,
            tc.tile_pool(name="tbl", bufs=1) as tblpool,
            tc.tile_pool(name="xin", bufs=2) as xpool,
            tc.tile_pool(name="idx", bufs=1) as ipool,
            tc.tile_pool(name="ohp", bufs=2) as ohpool,
            tc.tile_pool(name="gat", bufs=3) as gpool,
            tc.tile_pool(name="hbuf", bufs=3) as hpool,
            tc.tile_pool(name="stage", bufs=2) as spool,
            tc.tile_pool(name="psA", bufs=3, space="PSUM") as papool,
            tc.tile_pool(name="psT", bufs=1, space="PSUM") as ptpool,
            tc.tile_pool(name="ps2", bufs=3, space="PSUM") as p2pool,
        ):
            # ---- constants ----
            w1nc = cpool.tile([H, H], BF16, tag="w1nc")
            nc.sync.dma_start(w1nc[:], w1[0:H, :])
            w1pr = cpool.tile([H, H], BF16, tag="w1pr")
            nc.sync.dma_start(w1pr[:], w1[H:2 * H, :])
            b1_sb = cpool.tile([H, 1], F32, tag="b1")
            nc.sync.dma_start(b1_sb[:], b1[:])
            w2_sb = cpool.tile([H, 1], BF16, tag="w2")
            nc.sync.dma_start(w2_sb[:], w2[:])
            b2_sb = cpool.tile([1, 1], F32, tag="b2")
            nc.sync.dma_start(b2_sb[:], b2[:])

            # ---- indices for the B gather ----
            # First window's slice is DMAed separately so gather 0 can
            # start without waiting for the full index replication.
            idx_sb = ipool.tile([128, cols // 16], I16, tag="idx1")
            ic0 = T_WIN // 16
            for k in range(8):
                nc.sync.dma_start(idx_sb[16 * k:16 * (k + 1), 0:ic0],
                                  idx1[:, 0:ic0])
            for k in range(8):
                nc.sync.dma_start(idx_sb[16 * k:16 * (k + 1), ic0:],
                                  idx1[:, ic0:])

            # ---- A node table (ncRNA side, post-fc1) ----
            # node-major [128 lane, block, feat]; node n -> lane n%128,
            # block n//128.  Built as psum = x_chunk^T-stationary @ w1nc.
            a_sb = tblpool.tile([128, N_BLOCKS * H], BF16, tag="a_tbl")
            a_v = a_sb[:].rearrange("p (s f) -> p s f", s=N_BLOCKS)
            # zero first: the last node block is partial, and uninitialized
            # lanes would otherwise feed NaN*0 into matmuls.
            nc.vector.memset(a_sb[:], 0.0)

            for s in range(N_BLOCKS):
                lo = s * 128
                hi = min(N_NODES, lo + 128)
                xs = xpool.tile([H, 128], BF16, tag="xa")
                nc.sync.dma_start(xs[:, 0:hi - lo], xnc_t[:, lo:hi])
                pt = ptpool.tile([128, H], F32, tag="ptbl")
                nc.tensor.matmul(pt[0:hi - lo, :], xs[:, 0:hi - lo],
                                 w1nc[:], start=True, stop=True)
                if s % 2 == 0:
                    nc.scalar.activation(a_v[0:hi - lo, s, :],
                                         pt[0:hi - lo, :], ident)
                else:
                    nc.vector.tensor_scalar_add(a_v[0:hi - lo, s, :],
                                                pt[0:hi - lo, :], 0.0)

            # ---- edge loop ----
            ic = T_WIN // 16
            for _ in range(reps):
                for w in range(n_win):
                    c0 = w * T_WIN
                    gB = gpool.tile([H, T_WIN], BF16, tag="gB")
                    nc.gpsimd.dma_gather(
                        gB[:].rearrange("p (one t) -> p one t", one=1),
                        x_pr,
                        idx_sb[:, w * ic:(w + 1) * ic],
                        T_WIN,
                        T_WIN,
                        H,
                        transpose=True,
                        single_packet=False,
                    )
                    oh_sb = ohpool.tile([128, T_WIN], BF16, tag="oh")
                    nc.sync.dma_start(oh_sb[:], oh[:, c0:c0 + T_WIN])

                    stage = spool.tile([1, T_WIN], F32, tag="stage")
                    for k in range(T_WIN // CHUNK):
                        sl = slice(k * CHUNK, (k + 1) * CHUNK)
                        pa = papool.tile([128, CHUNK], F32, tag="psA")
                        segs = segs_per_tile[(c0 + k * CHUNK) // CHUNK]
                        # B side first: psA = W1pr^T @ x_pr[i1] (full width),
                        # then the A-side one-hot matmuls accumulate onto it.
                        nc.tensor.matmul(
                            pa[:], w1pr[:], gB[:, sl],
                            start=True, stop=False,
                        )
                        for j, (lo, hi, blk) in enumerate(segs):
                            nc.tensor.matmul(
                                pa[:, lo:hi],
                                a_v[:, blk, :],
                                oh_sb[:, k * CHUNK + lo:k * CHUNK + hi],
                                start=False, stop=(j == len(segs) - 1),
                            )
                        # h = relu(psA + b1) -> bf16
                        h = hpool.tile([128, CHUNK], BF16, tag="h")
                        nc.scalar.activation(h[:], pa[:], relu, bias=b1_sb[:])

                        p2 = p2pool.tile([1, CHUNK], F32, tag="ps2")
                        nc.tensor.matmul(p2[:], w2_sb[:], h[:],
                                         start=True, stop=True)
                        if k % 2 == 0:
                            nc.vector.tensor_scalar(
                                stage[:, sl], p2[:], b2_sb[:], None, add_op
                            )
                        else:
                            nc.scalar.activation(stage[:, sl], p2[:], ident,
                                                 bias=b2_sb[:])
                    nc.sync.dma_start(out[:, c0:c0 + T_WIN], stage[:])

    nc.compile()
    return nc


# ---------------------------------------------------------------------------
# Host-side wrapper
# ---------------------------------------------------------------------------

_CACHE: dict = {}


def _get_program(caps):
    key = tuple(caps)
    if key not in _CACHE:
        _CACHE[key] = _build(key)
    return _CACHE[key]


def _wrap16(idx: np.ndarray) -> np.ndarray:
    """int16 [16, n//16] with value i at [i % 16, i // 16]."""
    n = idx.shape[0]
    return np.ascontiguousarray(idx.astype(np.int16).reshape(n // 16, 16).T)


def kernel(
    x_ncRNA: np.ndarray,
    x_Protein: np.ndarray,
    edge_label_index: np.ndarray,
    W1: np.ndarray,
    b1: np.ndarray,
    W2: np.ndarray,
    b2: np.ndarray,
    _trace: bool = False,
) -> np.ndarray:
    E = edge_label_index.shape[1]

    i0 = np.asarray(edge_label_index[0], dtype=np.int64)
    i1 = np.asarray(edge_label_index[1], dtype=np.int64)

    # --- global bucket sort by i0 block, dealt round-robin across cores ---
    # Every bucket's edges are spread evenly over the 8 cores, so the shared
    # per-bucket capacity is ceil(n_b/8) and padding is minimal.
    blk = i0 >> 7
    order = np.argsort(blk, kind="stable")          # global edge ids, sorted
    n_b = np.bincount(blk, minlength=N_BLOCKS)
    bstart = np.concatenate([[0], np.cumsum(n_b)])
    pos = np.arange(E) - np.repeat(bstart[:-1], n_b)  # rank within bucket
    core_of = pos % N_CORES
    rank = pos // N_CORES

    caps = -(-n_b // N_CORES)
    total = int(caps.sum())
    caps[-1] += -total % T_WIN  # keep sum a multiple of T_WIN
    caps = tuple(int(x) for x in caps)
    cols = int(sum(caps))
    bases = np.concatenate([[0], np.cumsum(caps)]).astype(np.int64)
    col = np.repeat(bases[:-1], n_b) + rank         # column for sorted edges

    nc = _get_program(caps)

    xnc_t = np.ascontiguousarray(
        np.asarray(x_ncRNA, np.float32).T.astype(ml_dtypes.bfloat16))
    x_pr = np.ascontiguousarray(
        np.asarray(x_Protein).astype(ml_dtypes.bfloat16))
    w1 = np.ascontiguousarray(np.asarray(W1).astype(ml_dtypes.bfloat16))
    w2 = np.ascontiguousarray(np.asarray(W2).astype(ml_dtypes.bfloat16))
    b1_ = np.ascontiguousarray(np.asarray(b1).reshape(H, 1).astype(np.float32))
    b2_ = np.ascontiguousarray(np.asarray(b2).reshape(1, 1).astype(np.float32))

    in_maps = []
    placement = []  # per core: (global edge ids, their columns)
    for c in range(N_CORES):
        m = core_of == c
        ids_c = order[m]
        cols_c = col[m]

        # one-hot [128, cols] bf16
        oh = np.zeros((128, cols), ml_dtypes.bfloat16)
        oh[i0[ids_c] & 127, cols_c] = 1

        # B-side indices in column order (pad -> node 0)
        idxs = np.zeros(cols, np.int64)
        idxs[cols_c] = i1[ids_c]

        placement.append((ids_c, cols_c))
        in_maps.append({
            "xnc_t": xnc_t, "x_pr": x_pr,
            "w1": w1, "b1": b1_, "w2": w2, "b2": b2_,
            "oh": oh,
            "idx1": _wrap16(idxs),
        })

    res = bass_utils.run_bass_kernel_spmd(
        nc, in_maps, core_ids=list(range(N_CORES)), trace=_trace
    )
    out = np.empty(E, np.float32)
    for c, (ids_c, cols_c) in enumerate(placement):
        out[ids_c] = res.results[c]["out"][0][cols_c]
    kernel._last_results = res
    return out


# revision 21
# speedup vs baseline: 1.0258x; 1.0016x over previous
"""Trainium2 Bass kernel for nn_Classifier (GNN edge-MLP link predictor).

Computes, for E candidate edges:
    out[e] = W2 . relu( x_nc[i0[e]] @ W1[:H] + x_pr[i1[e]] @ W1[H:] + b1 ) + b2

Measured bottleneck of the naive kernel: Pool-engine (SWDGE) descriptor
generation for dma_gather costs ~9-10 ns/index in every mode, and the naive
kernel pays 2 gathered indices per edge.  This kernel pays only 1:

  - fc1 is hoisted to the node tables: A = Xnc @ W1[:H], B = Xpr @ W1[H:]
    are computed on device (157 small matmuls each) and kept in SBUF
    node-major as [128 lanes, 157 blocks, 128 feat] bf16.
  - Edges are sharded contiguously across 8 cores.  Per core, the host
    sorts its edges by i0 block (128-node blocks -> 157 buckets, padded to
    static per-bucket caps so the compiled program is input-independent).
  - A-side "gather" is a matmul: stationary = A_block, moving = a
    host-built one-hot (derived from indices only), PSUM gets A[i0[e]]
    feature-major.  ~1 PE cycle per edge instead of a Pool descriptor.
  - B-side is a single Pool dma_gather per 4096-edge window (SBUF-source,
    transposed, bf16) - the unavoidable 1 index/edge.
  - combine: DVE add (PSUM+SBUF), ACT relu(+b1) -> bf16, fc2 matmul
    (stationary w2), DVE/ACT drain (+b2), DMA out.
  - Host maps the bucket-padded column order back to edge order.

All matmuls bf16 with fp32 PSUM.
"""

import numpy as np
import ml_dtypes

import concourse.bass as bass
import concourse.tile as tile
from concourse import bacc, mybir
from concourse import bass_utils

F32 = mybir.dt.float32
BF16 = mybir.dt.bfloat16
I16 = mybir.dt.int16

N_CORES = 8
H = 128
N_NODES = 20000
N_BLOCKS = (N_NODES + 127) // 128  # 157

T_WIN = 4096    # edge columns per gather window
CHUNK = 512     # columns per PSUM tile


def _build(caps: tuple, reps: int = 1):
    """caps[b] = static column capacity of bucket b (multiples of 128,
    summing to a multiple of T_WIN)."""
    cols = int(sum(caps))
    assert cols % T_WIN == 0
    n_win = cols // T_WIN

    # For every CHUNK-wide psum tile, the static list of
    # (lo, hi, block) segments it covers.
    bases = np.concatenate([[0], np.cumsum(caps)]).astype(int)
    segs_per_tile = []
    for t0 in range(0, cols, CHUNK):
        t1 = t0 + CHUNK
        segs = []
        for b in range(len(caps)):
            lo, hi = max(t0, bases[b]), min(t1, bases[b + 1])
            if lo < hi:
                segs.append((lo - t0, hi - t0, b % N_BLOCKS))
        segs_per_tile.append(segs)

    nc = bacc.Bacc("TRN2", target_bir_lowering=False, debug=False,
                   num_devices=N_CORES)

    xnc_t = nc.dram_tensor("xnc_t", [H, N_NODES], BF16, kind="ExternalInput").ap()
    x_pr = nc.dram_tensor("x_pr", [N_NODES, H], BF16, kind="ExternalInput").ap()
    w1 = nc.dram_tensor("w1", [2 * H, H], BF16, kind="ExternalInput").ap()
    b1 = nc.dram_tensor("b1", [H, 1], F32, kind="ExternalInput").ap()
    w2 = nc.dram_tensor("w2", [H, 1], BF16, kind="ExternalInput").ap()
    b2 = nc.dram_tensor("b2", [1, 1], F32, kind="ExternalInput").ap()
    oh = nc.dram_tensor("oh", [128, cols], BF16, kind="ExternalInput").ap()
    idx1 = nc.dram_tensor("idx1", [16, cols // 16], I16, kind="ExternalInput").ap()
    out = nc.dram_tensor("out", [1, cols], F32, kind="ExternalOutput").ap()

    relu = mybir.ActivationFunctionType.Relu
    ident = mybir.ActivationFunctionType.Identity
    add_op = mybir.AluOpType.add

    with tile.TileContext(nc) as tc:
        with (
            tc.tile_pool(name="const", bufs=1) as cpool,
            tc.tile_pool(name="tbl", bufs=1) as tblpool,
            tc.tile_pool(name="xin", bufs=2) as xpool,
            tc.tile_pool(name="idx", bufs=1) as ipool,
            tc.tile_pool(name="ohp", bufs=2) as ohpool,
            tc.tile_pool(name="gat", bufs=3) as gpool,
            tc.tile_pool(name="hbuf", bufs=3) as hpool,
            tc.tile_pool(name="stage", bufs=2) as spool,
            tc.tile_pool(name="psA", bufs=3, space="PSUM") as papool,
            tc.tile_pool(name="psT", bufs=1, space="PSUM") as ptpool,
            tc.tile_pool(name="ps2", bufs=3, space="PSUM") as p2pool,
        ):
            # ---- indices for the B gather (first window first, so gather 0
            # unblocks before the const/table DMAs queue up) ----
            idx_sb = ipool.tile([128, cols // 16], I16, tag="idx1")
            ic0 = T_WIN // 16
            for k in range(8):
                nc.sync.dma_start(idx_sb[16 * k:16 * (k + 1), 0:ic0],
                                  idx1[:, 0:ic0])

            # ---- constants ----
            w1nc = cpool.tile([H, H], BF16, tag="w1nc")
            nc.sync.dma_start(w1nc[:], w1[0:H, :])
            w1pr = cpool.tile([H, H], BF16, tag="w1pr")
            nc.sync.dma_start(w1pr[:], w1[H:2 * H, :])
            b1_sb = cpool.tile([H, 1], F32, tag="b1")
            nc.sync.dma_start(b1_sb[:], b1[:])
            w2_sb = cpool.tile([H, 1], BF16, tag="w2")
            nc.sync.dma_start(w2_sb[:], w2[:])
            b2_sb = cpool.tile([1, 1], F32, tag="b2")
            nc.sync.dma_start(b2_sb[:], b2[:])

            # ---- remaining index replication ----
            for k in range(8):
                nc.sync.dma_start(idx_sb[16 * k:16 * (k + 1), ic0:],
                                  idx1[:, ic0:])

            # ---- A node table (ncRNA side, post-fc1) ----
            # node-major [128 lane, block, feat]; node n -> lane n%128,
            # block n//128.  Built as psum = x_chunk^T-stationary @ w1nc.
            a_sb = tblpool.tile([128, N_BLOCKS * H], BF16, tag="a_tbl")
            a_v = a_sb[:].rearrange("p (s f) -> p s f", s=N_BLOCKS)
            # zero first: the last node block is partial, and uninitialized
            # lanes would otherwise feed NaN*0 into matmuls.
            nc.vector.memset(a_sb[:], 0.0)

            for s in range(N_BLOCKS):
                lo = s * 128
                hi = min(N_NODES, lo + 128)
                xs = xpool.tile([H, 128], BF16, tag="xa")
                nc.sync.dma_start(xs[:, 0:hi - lo], xnc_t[:, lo:hi])
                pt = ptpool.tile([128, H], F32, tag="ptbl")
                nc.tensor.matmul(pt[0:hi - lo, :], xs[:, 0:hi - lo],
                                 w1nc[:], start=True, stop=True)
                if s % 2 == 0:
                    nc.scalar.activation(a_v[0:hi - lo, s, :],
                                         pt[0:hi - lo, :], ident)
                else:
                    nc.vector.tensor_scalar_add(a_v[0:hi - lo, s, :],
                                                pt[0:hi - lo, :], 0.0)

            # ---- edge loop ----
            ic = T_WIN // 16
            for _ in range(reps):
                for w in range(n_win):
                    c0 = w * T_WIN
                    gB = gpool.tile([H, T_WIN], BF16, tag="gB")
                    nc.gpsimd.dma_gather(
                        gB[:].rearrange("p (one t) -> p one t", one=1),
                        x_pr,
                        idx_sb[:, w * ic:(w + 1) * ic],
                        T_WIN,
                        T_WIN,
                        H,
                        transpose=True,
                        single_packet=False,
                    )
                    oh_sb = ohpool.tile([128, T_WIN], BF16, tag="oh")
                    nc.sync.dma_start(oh_sb[:], oh[:, c0:c0 + T_WIN])

                    stage = spool.tile([1, T_WIN], F32, tag="stage")
                    for k in range(T_WIN // CHUNK):
                        sl = slice(k * CHUNK, (k + 1) * CHUNK)
                        pa = papool.tile([128, CHUNK], F32, tag="psA")
                        segs = segs_per_tile[(c0 + k * CHUNK) // CHUNK]
                        # B side first: psA = W1pr^T @ x_pr[i1] (full width),
                        # then the A-side one-hot matmuls accumulate onto it.
                        nc.tensor.matmul(
                            pa[:], w1pr[:], gB[:, sl],
                            start=True, stop=False,
                        )
                        for j, (lo, hi, blk) in enumerate(segs):
                            nc.tensor.matmul(
                                pa[:, lo:hi],
                                a_v[:, blk, :],
                                oh_sb[:, k * CHUNK + lo:k * CHUNK + hi],
                                start=False, stop=(j == len(segs) - 1),
                            )
                        # h = relu(psA + b1) -> bf16
                        h = hpool.tile([128, CHUNK], BF16, tag="h")
                        nc.scalar.activation(h[:], pa[:], relu, bias=b1_sb[:])

                        p2 = p2pool.tile([1, CHUNK], F32, tag="ps2")
                        nc.tensor.matmul(p2[:], w2_sb[:], h[:],
                                         start=True, stop=True)
                        if k % 2 == 0:
                            nc.vector.tensor_scalar(
                                stage[:, sl], p2[:], b2_sb[:], None, add_op
                            )
                        else:
                            nc.scalar.activation(stage[:, sl], p2[:], ident,
                                                 bias=b2_sb[:])
                    nc.sync.dma_start(out[:, c0:c0 + T_WIN], stage[:])

    nc.compile()
    return nc


# ---------------------------------------------------------------------------
# Host-side wrapper
# ---------------------------------------------------------------------------

_CACHE: dict = {}


def _get_program(caps):
    key = tuple(caps)
    if key not in _CACHE:
        _CACHE[key] = _build(key)
    return _CACHE[key]


def _wrap16(idx: np.ndarray) -> np.ndarray:
    """int16 [16, n//16] with value i at [i % 16, i // 16]."""
    n = idx.shape[0]
    return np.ascontiguousarray(idx.astype(np.int16).reshape(n // 16, 16).T)


def kernel(
    x_ncRNA: np.ndarray,
    x_Protein: np.ndarray,
    edge_label_index: np.ndarray,
    W1: np.ndarray,
    b1: np.ndarray,
    W2: np.ndarray,
    b2: np.ndarray,
    _trace: bool = False,
) -> np.ndarray:
    E = edge_label_index.shape[1]

    i0 = np.asarray(edge_label_index[0], dtype=np.int64)
    i1 = np.asarray(edge_label_index[1], dtype=np.int64)

    # --- global bucket sort by i0 block, dealt round-robin across cores ---
    # Every bucket's edges are spread evenly over the 8 cores, so the shared
    # per-bucket capacity is ceil(n_b/8) and padding is minimal.
    blk = i0 >> 7
    order = np.argsort(blk, kind="stable")          # global edge ids, sorted
    n_b = np.bincount(blk, minlength=N_BLOCKS)
    bstart = np.concatenate([[0], np.cumsum(n_b)])
    pos = np.arange(E) - np.repeat(bstart[:-1], n_b)  # rank within bucket
    core_of = pos % N_CORES
    rank = pos // N_CORES

    caps = -(-n_b // N_CORES)
    total = int(caps.sum())
    caps[-1] += -total % T_WIN  # keep sum a multiple of T_WIN
    caps = tuple(int(x) for x in caps)
    cols = int(sum(caps))
    bases = np.concatenate([[0], np.cumsum(caps)]).astype(np.int64)
    col = np.repeat(bases[:-1], n_b) + rank         # column for sorted edges

    nc = _get_program(caps)

    xnc_t = np.ascontiguousarray(
        np.asarray(x_ncRNA, np.float32).T.astype(ml_dtypes.bfloat16))
    x_pr = np.ascontiguousarray(
        np.asarray(x_Protein).astype(ml_dtypes.bfloat16))
    w1 = np.ascontiguousarray(np.asarray(W1).astype(ml_dtypes.bfloat16))
    w2 = np.ascontiguousarray(np.asarray(W2).astype(ml_dtypes.bfloat16))
    b1_ = np.ascontiguousarray(np.asarray(b1).reshape(H, 1).astype(np.float32))
    b2_ = np.ascontiguousarray(np.asarray(b2).reshape(1, 1).astype(np.float32))

    in_maps = []
    placement = []  # per core: (global edge ids, their columns)
    for c in range(N_CORES):
        m = core_of == c
        ids_c = order[m]
        cols_c = col[m]

        # one-hot [128, cols] bf16
        oh = np.zeros((128, cols), ml_dtypes.bfloat16)
        oh[i0[ids_c] & 127, cols_c] = 1

        # B-side indices in column order (pad -> node 0)
        idxs = np.zeros(cols, np.int64)
        idxs[cols_c] = i1[ids_c]

        placement.append((ids_c, cols_c))
        in_maps.append({
            "xnc_t": xnc_t, "x_pr": x_pr,
            "w1": w1, "b1": b1_, "w2": w2, "b2": b2_,
            "oh": oh,
            "idx1": _wrap16(idxs),
        })

    res = bass_utils.run_bass_kernel_spmd(
        nc, in_maps, core_ids=list(range(N_CORES)), trace=_trace
    )
    out = np.empty(E, np.float32)
    for c, (ids_c, cols_c) in enumerate(placement):
        out[ids_c] = res.results[c]["out"][0][cols_c]
    kernel._last_results = res
    return out
